# revision 1
# baseline (speedup 1.0000x reference)
# Bass/Tile Trainium2 kernel for batched multi-head causal self-attention.
#
# Problem: x[B=2,T=2048,C=1024], 16 heads (hd=64), causal softmax attention,
# output projection. Full (unsharded) inputs in, full output out.
#
# Sharding (Megatron-style): 8 cores = 2 batch groups x 4 head groups.
# Core i handles batch b = i // 4 and heads [4*(i%4) : 4*(i%4)+4).
# Each core computes Q/K/V projections for its 4 heads, causal attention,
# and a partial output projection (contribution of its heads).  The host
# sums the 4 partials per batch (the Megatron all-reduce) and adds bias.
#
# On-device layout notes:
#   - Everything is kept "transposed" (feature dim on partitions):
#     xT [C, T], QT/KT [64, T] per head.  Heads come in pairs packed on
#     the 128 partitions (even head at [0:64], odd head at [64:128]); the
#     K=64 S^T matmuls of a pair use explicit tile_position row groups so
#     they can run concurrently on disjoint PE quadrants.
#   - V is stored [T, 64] per head augmented with a ones column (V') so
#     the P@V matmul also produces the softmax denominator (row 64).
#   - Softmax runs without max-subtraction (scores are bounded ~|10|, exp
#     is safe in fp32), so no partition-dim reductions are ever needed.
#   - Causal masking: k-tiles strictly above the diagonal are skipped;
#     tiles crossing the diagonal get a 128x128 triangular mask multiply
#     and a column-restricted P@V matmul.
#   - QKV matmuls and all attention internals (x, Wq/Wk/Wv, QT/KT/V'/P^T)
#     are bf16; the normalized O^T and Wp stay fp32 and the projection
#     runs in float32r (full fp32 data at ~full PE rate).
#   - Softmax denominators: per-chain rows are DMA'd into a [8, 512]
#     collection tile (DMA may write any partition; engines may not), one
#     batched DVE reciprocal serves a whole head-pair, and GpSimd
#     partition-broadcast + DVE multiply apply the normalization.

import numpy as np

import concourse.bass as bass
import concourse.tile as tile
from concourse import bacc, mybir
from concourse import bass_utils

F32 = mybir.dt.float32
F32R = mybir.dt.float32r
BF16 = mybir.dt.bfloat16
ATT_DT = BF16   # dtype of attention operands (qt/kt/v'/pt/mask)

B, T, C, H = 2, 2048, 1024, 16
HD = C // H            # 64 head dim
NCORES = 8
HPC = 4                # heads per core
DSEL = HPC * HD        # 256 feature dims per core
NTT = T // 128         # 16 t-tiles of 128
NTB = T // 512         # 4 t-blocks of 512
NCC = C // 128         # 8 c-chunks of 128
NQB = T // 512         # 4 q-blocks of 512


def build_program(do_attn=True, do_proj=True, attn_sel=None, dump_ot=False):
    nc = bacc.Bacc("TRN2", target_bir_lowering=False, debug=False)

    # host-prepared "SBUF images": [128 partitions, ...] with long
    # contiguous per-partition lines for efficient DMA
    xT = nc.dram_tensor("xT", [128, NCC, T], BF16, kind="ExternalInput").ap()
    wqT = nc.dram_tensor("wqT", [128, NCC * DSEL], BF16, kind="ExternalInput").ap()
    wkT = nc.dram_tensor("wkT", [128, NCC * DSEL], BF16, kind="ExternalInput").ap()
    wvT = nc.dram_tensor("wvT", [128, NCC * DSEL], BF16, kind="ExternalInput").ap()
    wpT = nc.dram_tensor("wpT", [128, 2 * C], F32R, kind="ExternalInput").ap()
    maskd = nc.dram_tensor("maskd", [128, 128], ATT_DT, kind="ExternalInput").ap()
    out_p = nc.dram_tensor("out_p", [T, C], F32, kind="ExternalOutput").ap()


    with tile.TileContext(nc) as tc:
        with (
            tc.tile_pool(name="consts", bufs=1) as consts,
            tc.tile_pool(name="persist", bufs=1) as persist,
            tc.tile_pool(name="xin", bufs=10) as xin,
            tc.tile_pool(name="pt", bufs=8) as ptpool,
            tc.tile_pool(name="norm", bufs=12) as norm,
            tc.tile_pool(name="outst", bufs=4) as outst,
            tc.tile_pool(name="pa", bufs=4, space="PSUM") as pa,
        ):
            # ---- constants / weights -------------------------------------
            wq_sb = consts.tile([128, NCC, DSEL], BF16, tag="wq")
            wk_sb = consts.tile([128, NCC, DSEL], BF16, tag="wk")
            wv_sb = consts.tile([128, NCC, DSEL], BF16, tag="wv")
            wp_sb = consts.tile([128, 2, C], F32R, tag="wp")
            mk_sb = consts.tile([128, 128], ATT_DT, tag="mk")
            xt_first = xin.tile([128, 1024], BF16, tag="xt", name="xt_first")
            for pg in range(4):
                nc.sync.dma_start(out=xt_first[32 * pg : 32 * pg + 32, :],
                                  in_=xT[32 * pg : 32 * pg + 32, 0, 0:1024])
            for pg in range(4):
                pgs = slice(32 * pg, 32 * pg + 32)
                nc.sync.dma_start(
                    out=wq_sb[pgs].rearrange("p cc d -> p (cc d)"), in_=wqT[pgs])
            for pg in range(4):
                pgs = slice(32 * pg, 32 * pg + 32)
                nc.sync.dma_start(
                    out=wk_sb[pgs].rearrange("p cc d -> p (cc d)"), in_=wkT[pgs])
                nc.sync.dma_start(
                    out=wv_sb[pgs].rearrange("p cc d -> p (cc d)"), in_=wvT[pgs])

            # ---- persistent activations ----------------------------------
            # QT/KT/OT: head pairs packed on partitions ([0:64] even slot,
            # [64:128] odd slot), free dim = t
            qt_sb = persist.tile([128, 2, T], ATT_DT, tag="qt")
            kt_sb = persist.tile([128, 2, T], ATT_DT, tag="kt")
            ot_sb = persist.tile([128, 2, T], F32R, tag="ot")
            # V' per k-tile: 4 heads x (64 V cols + 1 ones col)
            v_sb = persist.tile([128, NTT, HPC * (HD + 1)], ATT_DT, tag="v")

            ones_sb = consts.tile([128, NTT], F32, tag="ones")
            nc.vector.memset(ones_sb[:], 1.0)
            for h in range(HPC):
                nc.vector.tensor_copy(
                    out=v_sb[:, :, h * 65 + 64 : h * 65 + 65],
                    in_=ones_sb[:].rearrange("p (t o) -> p t o", o=1),
                )

            # ---- phase 1: QKV projections --------------------------------
            # QT[d, t] = sum_c wqT[c, d] * xT[c, t]   (and same for K)
            # V[t, d]  = sum_c xT[c, t] * wvT[c, d]
            for tbp in range(NTB // 2):
                xts = []
                for cc in range(NCC):
                    if tbp == 0 and cc == 0:
                        xts.append(xt_first)
                        continue
                    xt = xin.tile([128, 1024], BF16, tag="xt", name=f"xt{cc}")
                    tsp = slice(tbp * 1024, tbp * 1024 + 1024)
                    nc.sync.dma_start(out=xt[0:64, :], in_=xT[0:64, cc, tsp])
                    nc.sync.dma_start(out=xt[64:128, :], in_=xT[64:128, cc, tsp])
                    xts.append(xt)
                for ti in range(2):
                    tb = 2 * tbp + ti
                    ts = slice(tb * 512, tb * 512 + 512)
                    tsl2 = slice(ti * 512, ti * 512 + 512)
                    pq = pa.tile([128, 1024], F32, tag="pa", name="pq")
                    pk = pa.tile([128, 1024], F32, tag="pa", name="pk")
                    pv = pa.tile([128, 1024], F32, tag="pa", name="pv")
                    for cc in range(NCC):
                        xt = xts[cc]
                        st = dict(start=(cc == 0), stop=(cc == NCC - 1))
                        nc.tensor.matmul(pq[:, 0:512], wq_sb[:, cc, 0:128], xt[:, tsl2], **st)
                        nc.tensor.matmul(pq[:, 512:1024], wq_sb[:, cc, 128:256], xt[:, tsl2], **st)
                        nc.tensor.matmul(pk[:, 0:512], wk_sb[:, cc, 0:128], xt[:, tsl2], **st)
                        nc.tensor.matmul(pk[:, 512:1024], wk_sb[:, cc, 128:256], xt[:, tsl2], **st)
                        for tt in range(4):
                            # two 256-col regions share a PSUM bank: only the
                            # first toucher of a bank may set start, only the
                            # last may set stop
                            nc.tensor.matmul(
                                pv[:, tt * 256 : tt * 256 + 256],
                                xt[:, ti * 512 + tt * 128 : ti * 512 + tt * 128 + 128],
                                wv_sb[:, cc, :],
                                start=(cc == 0 and tt % 2 == 0),
                                stop=(cc == NCC - 1 and tt % 2 == 1),
                            )
                    # PSUM -> SBUF (casts to bf16)
                    nc.vector.tensor_copy(
                        out=qt_sb[:, :, ts], in_=pq[:].rearrange("p (s t) -> p s t", s=2)
                    )
                    nc.vector.tensor_copy(
                        out=kt_sb[:, :, ts], in_=pk[:].rearrange("p (s t) -> p s t", s=2)
                    )
                    pv3 = pv[:].rearrange("p (tt d) -> p tt d", tt=4)
                    for h in range(HPC):
                        nc.vector.tensor_copy(
                            out=v_sb[:, tb * 4 : tb * 4 + 4, h * 65 : h * 65 + 64],
                            in_=pv3[:, :, h * 64 : h * 64 + 64],
                        )

            # wp / mask are not needed until later phases: issue their DMAs
            # after phase 1 so they don't delay the first matmuls
            nc.sync.dma_start(out=mk_sb[:], in_=maskd)
            for pg in range(4):
                pgs = slice(32 * pg, 32 * pg + 32)
                nc.sync.dma_start(
                    out=wp_sb[pgs].rearrange("p h c -> p (h c)"), in_=wpT[pgs])

            # ---- phase 2: attention per (head-pair, q-block) -------------
            # S^T[k, q] tiles via K=64 matmuls (pair slots concurrent on PE),
            # exp on ACT, diag-block masking on DVE, P@V' accumulation on PE.
            scale = 1.0 / float(np.sqrt(HD))
            attn = [(hp, qb) for qb in reversed(range(NQB)) for hp in range(2)]
            if not do_attn:
                attn = []
            if attn_sel is not None:
                attn = attn_sel
            den = [persist.tile([4, 512], F32, tag=f"den{i}", name=f"den{i}")
                   for i in range(NQB)]
            rec = [persist.tile([4, 512], F32, tag=f"rec{i}", name=f"rec{i}")
                   for i in range(NQB)]
            psq = {}

            def proj_block(qb):
                for tt in range(4 * qb, 4 * qb + 4):
                    tloc = slice(tt * 128, tt * 128 + 128)
                    pc = pa.tile([128, 1024], F32, tag="pa", name="pc")
                    for cb in range(2):
                        for hpp in range(2):
                            nc.tensor.matmul(
                                pc[:, cb * 512 : cb * 512 + 512],
                                ot_sb[:, hpp, tloc],
                                wp_sb[:, hpp, cb * 512 : cb * 512 + 512],
                                start=(hpp == 0),
                                stop=(hpp == 1),
                            )
                    ob = outst.tile([128, 1024], F32, tag="ob")
                    if tt % 2 == 0:
                        nc.vector.tensor_copy(out=ob[:], in_=pc[:])
                    else:
                        nc.scalar.copy(ob[:], pc[:])
                    for pg in range(4):
                        eng = nc.sync if (tt + pg) % 2 else nc.scalar
                        eng.dma_start(
                            out=out_p[tt * 128 + 32 * pg : tt * 128 + 32 * pg + 32, :],
                            in_=ob[32 * pg : 32 * pg + 32, :])
            for hp, qb in attn:
                qs = slice(qb * 512, qb * 512 + 512)
                n_kt = 4 * (qb + 1)          # k-tiles (128) up to diagonal
                n_g = n_kt // 2              # groups of 2 k-tiles
                po = pa.tile([128, 1024], F32, tag="pa", name="po")
                for g in range(n_g):
                    sg = [pa.tile([128, 1024], F32, tag="pa", name=f"sg{s}")
                          for s in range(2)]
                    pt = [ptpool.tile([128, 1024], ATT_DT, tag="pt", name=f"pt{s}")
                          for s in range(2)]
                    for s in range(2):   # slot-major: exp(s) can start
                        psl = slice(64 * s, 64 * s + 64)
                        for sl in range(2):
                            kt = 2 * g + sl
                            nc.tensor.matmul(
                                sg[s][:, sl * 512 : sl * 512 + 512],
                                kt_sb[psl, hp, kt * 128 : kt * 128 + 128],
                                qt_sb[psl, hp, qs],
                                start=True,
                                stop=True,
                                tile_position=(64 * s, 0),
                            )
                        # exp (no max subtraction; scores bounded)
                        nc.scalar.activation(
                            out=pt[s][:], in_=sg[s][:],
                            func=mybir.ActivationFunctionType.Exp,
                            scale=scale,
                        )
                    for sl in range(2):
                        kt = 2 * g + sl
                        j = kt - 4 * qb      # diag offset, >=0 on diag group
                        roff = 128 * j if j >= 0 else 0
                        for s in range(2):
                            if j >= 0:
                                # triangular mask on the diagonal block
                                dcol = sl * 512 + 128 * j
                                nc.vector.tensor_mul(
                                    pt[s][:, dcol : dcol + 128],
                                    pt[s][:, dcol : dcol + 128],
                                    mk_sb[:],
                                )
                            h = 2 * hp + s
                            nc.tensor.matmul(
                                po[0:65, s * 512 + roff : s * 512 + 512],
                                v_sb[:, kt, h * 65 : h * 65 + 65],
                                pt[s][:, sl * 512 + roff : sl * 512 + 512],
                                start=(kt == 0),
                                stop=(kt == n_kt - 1),
                            )
                # copy O^T + denominator row out of PSUM; normalization is
                # deferred so one batched reciprocal serves the head-pair
                for s in range(2):
                    ps_sb = norm.tile([65, 512], F32, tag="ps",
                                      name=f"ps{hp}{qb}{s}")
                    nc.vector.tensor_copy(out=ps_sb[:], in_=po[0:65, s * 512 : s * 512 + 512])
                    # DMA may read/write any partition row (engines cannot)
                    idx = 2 * hp + s
                    nc.sync.dma_start(out=den[qb][idx : idx + 1, :],
                                      in_=ps_sb[64:65, :])
                    psq[(hp, qb, s)] = ps_sb
                if hp == 1:
                    nc.vector.reciprocal(rec[qb][:], den[qb][:])
                    for hp2 in range(2):
                        for s in range(2):
                            idx = 2 * hp2 + s
                            rc = norm.tile([1, 512], F32, tag="rc", name="rc")
                            rb = norm.tile([64, 512], F32, tag="rb", name="rb")
                            nc.sync.dma_start(out=rc[:], in_=rec[qb][idx : idx + 1, :])
                            nc.gpsimd.partition_broadcast(rb[:], rc[:])
                            nc.vector.tensor_mul(
                                ot_sb[64 * s : 64 * s + 64, hp2,
                                      qb * 512 : qb * 512 + 512],
                                psq[(hp2, qb, s)][0:64, :],
                                rb[:],
                            )

            for qb in reversed(range(NQB)):
                proj_block(qb)

            if dump_ot:
                nc.sync.dma_start(out=out_p[0:128, :],
                                  in_=ot_sb[:, 0, 0:1024].bitcast(F32))
                nc.sync.dma_start(out=out_p[128:256, :],
                                  in_=ot_sb[:, 1, 0:1024].bitcast(F32))

    nc.compile()
    return nc


_NC_CACHE = None


def _get_program():
    global _NC_CACHE
    if _NC_CACHE is None:
        _NC_CACHE = build_program()
    return _NC_CACHE


def make_in_maps(x, Wq, Wk, Wv, Wp):
    import ml_dtypes
    x = np.asarray(x, np.float32)
    Wq = np.asarray(Wq, np.float32)
    Wk = np.asarray(Wk, np.float32)
    Wv = np.asarray(Wv, np.float32)
    Wp = np.asarray(Wp, np.float32)
    maskd = np.triu(np.ones((128, 128), ml_dtypes.bfloat16))  # mask[k,q]=(k<=q)
    in_maps = []
    for core in range(NCORES):
        b, hg = core // 4, core % 4
        sel = slice(hg * DSEL, (hg + 1) * DSEL)
        # SBUF images: [128, cc, ...] with partition index innermost in
        # the original feature dim (feature c -> (cc, p))
        xi = x[b].T.reshape(NCC, 128, T).transpose(1, 0, 2)          # [128, cc, T]
        wqi = Wq[sel, :].T.reshape(NCC, 128, DSEL).transpose(1, 0, 2).reshape(128, NCC * DSEL)
        wki = Wk[sel, :].T.reshape(NCC, 128, DSEL).transpose(1, 0, 2).reshape(128, NCC * DSEL)
        wvi = Wv[sel, :].T.reshape(NCC, 128, DSEL).transpose(1, 0, 2).reshape(128, NCC * DSEL)
        wpi = Wp[:, sel].T.reshape(2, 128, C).transpose(1, 0, 2).reshape(128, 2 * C)
        in_maps.append({
            "xT": np.ascontiguousarray(xi.astype(ml_dtypes.bfloat16)),
            "wqT": np.ascontiguousarray(wqi.astype(ml_dtypes.bfloat16)),
            "wkT": np.ascontiguousarray(wki.astype(ml_dtypes.bfloat16)),
            "wvT": np.ascontiguousarray(wvi.astype(ml_dtypes.bfloat16)),
            "wpT": np.ascontiguousarray(wpi),
            "maskd": maskd,
        })
    return in_maps


def combine_outputs(results, bp):
    parts = [results[i]["out_p"] for i in range(NCORES)]
    out = np.stack([
        parts[0] + parts[1] + parts[2] + parts[3],
        parts[4] + parts[5] + parts[6] + parts[7],
    ])
    return (out + np.asarray(bp, np.float32)).astype(np.float32)


def kernel(x, Wq, Wk, Wv, Wp, bp):
    nc = _get_program()
    in_maps = make_in_maps(x, Wq, Wk, Wv, Wp)
    res = bass_utils.run_bass_kernel_spmd(nc, in_maps, core_ids=list(range(NCORES)))
    return combine_outputs(res.results, bp)



# revision 9
# speedup vs baseline: 1.0741x; 1.0741x over previous
# Bass/Tile Trainium2 kernel for batched multi-head causal self-attention.
#
# Problem: x[B=2,T=2048,C=1024], 16 heads (hd=64), causal softmax attention,
# output projection. Full (unsharded) inputs in, full output out.
#
# Sharding (Megatron-style): 8 cores = 2 batch groups x 4 head groups.
# Core i handles batch b = i // 4 and heads [4*(i%4) : 4*(i%4)+4).
# Each core computes Q/K/V projections for its 4 heads, causal attention,
# and a partial output projection (contribution of its heads).  The host
# sums the 4 partials per batch (the Megatron all-reduce) and adds bias.
#
# v2 structure (fully software-pipelined):
#   QKV(tb0); for qb: attn(hp0,qb), attn(hp1,qb), QKV(tb=qb+1), norm(qb),
#   proj(qb).  The attention exp work (ACT engine) overlaps the QKV/proj
#   matmuls of neighboring blocks, the normalization chain (DMA/DVE/GpSimd)
#   hides under QKV, and the output DMA is spread across the whole kernel.
#
# On-device layout notes:
#   - Feature dims on partitions: xT [C, T], QT/KT [64, T] per head, with
#     head pairs packed on the 128 partitions (even head at [0:64], odd at
#     [64:128]).  S^T tiles are built with K=64 matmuls using explicit
#     tile_position row groups; emitting the two slots back-to-back lets
#     them run concurrently on disjoint PE row halves.
#   - sg PSUM tile = [128 k, slot0 512q | slot1 512q] for ONE k-tile: one
#     [128,1024] exp per k-tile serves both heads of the pair and the
#     pipeline (depth 2 + po accumulator) exactly fills the 8 PSUM banks.
#   - V' = [V | ones] per head so the P@V matmul also yields the softmax
#     denominator (row 64).  P@V is split into two K=64 halves paired on
#     disjoint PE row groups (2x concurrency vs a single K=128 matmul).
#   - exp has no max-subtraction (scores bounded); on diagonal k-tiles the
#     exp AP skips the fully-masked column range (roff) and a [128,2,128]
#     strided multiply applies the triangular mask to both slots at once.
#   - Normalization: denominator rows collected per q-block, one
#     reciprocal_approx_fast [4,512], GpSimd partition-broadcast, DVE mul
#     into bf16 OT; scheduled while PE runs the next block's QKV.
#   - Projection runs in bf16 (lhsT=OT, rhs=Wp); partial outputs are cast
#     to fp16 and DMA'd out incrementally; host sums partials + bias.

import numpy as np

import concourse.bass as bass
import concourse.tile as tile
from concourse import bacc, mybir
from concourse import bass_utils

F32 = mybir.dt.float32
F16 = mybir.dt.float16
BF16 = mybir.dt.bfloat16
ATT_DT = BF16

B, T, C, H = 2, 2048, 1024, 16
HD = C // H            # 64 head dim
NCORES = 8
HPC = 4                # heads per core
DSEL = HPC * HD        # 256 feature dims per core
NTT = T // 128         # 16 t-tiles of 128
NCC = C // 128         # 8 c-chunks of 128
NQB = T // 512         # 4 q-blocks of 512


def build_program(pv_split=False, exp_trim=True, qt_act=True):
    nc = bacc.Bacc("TRN2", target_bir_lowering=False, debug=False)

    # host-prepared "SBUF images": [128 partitions, ...] with long
    # contiguous per-partition lines for efficient DMA
    xT = nc.dram_tensor("xT", [128, NCC, T], BF16, kind="ExternalInput").ap()
    wqT = nc.dram_tensor("wqT", [128, NCC * DSEL], BF16, kind="ExternalInput").ap()
    wkT = nc.dram_tensor("wkT", [128, NCC * DSEL], BF16, kind="ExternalInput").ap()
    wvT = nc.dram_tensor("wvT", [128, NCC * DSEL], BF16, kind="ExternalInput").ap()
    wpT = nc.dram_tensor("wpT", [128, 2 * C], BF16, kind="ExternalInput").ap()
    maskd = nc.dram_tensor("maskd", [128, 256], ATT_DT, kind="ExternalInput").ap()
    out_p = nc.dram_tensor("out_p", [T, C], F16, kind="ExternalOutput").ap()

    scale = 1.0 / float(np.sqrt(HD))

    # V' column stride per head: 65 = [V|ones]; 96 pads to a 32-multiple so
    # the K-split PV matmuls have 32-aligned col groups for tile_position
    VW = 96 if pv_split == 2 else 65

    with tile.TileContext(nc) as tc:
        with (
            tc.tile_pool(name="consts", bufs=1) as consts,
            tc.tile_pool(name="persist", bufs=1) as persist,
            tc.tile_pool(name="xin", bufs=16) as xin,
            tc.tile_pool(name="pt", bufs=6) as ptpool,
            tc.tile_pool(name="norm", bufs=12) as norm,
            tc.tile_pool(name="outst", bufs=4) as outst,
            tc.tile_pool(name="pa", bufs=3, space="PSUM") as pa,
            tc.tile_pool(name="pb", bufs=1, space="PSUM") as pb,
        ):
            # ---- constants / weights -------------------------------------
            wq_sb = consts.tile([128, NCC, DSEL], BF16, tag="wq")
            wk_sb = consts.tile([128, NCC, DSEL], BF16, tag="wk")
            wv_sb = consts.tile([128, NCC, DSEL], BF16, tag="wv")
            wp_sb = consts.tile([128, 2, C], BF16, tag="wp")
            mk_sb = consts.tile([128, 2, 128], ATT_DT, tag="mk")
            for pg in range(4):
                pgs = slice(32 * pg, 32 * pg + 32)
                nc.sync.dma_start(
                    out=wq_sb[pgs].rearrange("p cc d -> p (cc d)"), in_=wqT[pgs])

            # ---- persistent activations ----------------------------------
            qt_sb = persist.tile([128, 2, T], ATT_DT, tag="qt")
            kt_sb = persist.tile([128, 2, T], ATT_DT, tag="kt")
            ot_sb = persist.tile([128, 2, T], BF16, tag="ot")
            # V' per k-tile: 4 heads x (64 V cols + 1 ones col)
            v_sb = persist.tile([128, NTT, HPC * (HD + 1)], ATT_DT, tag="v")
            den = [persist.tile([4, 512], F32, tag=f"den{i}", name=f"den{i}")
                   for i in range(NQB)]
            rec = [persist.tile([4, 512], F32, tag=f"rec{i}", name=f"rec{i}")
                   for i in range(NQB)]

            # ---- x loads (per t-block of 512) ----------------------------
            def load_x(tb):
                ts_ = slice(tb * 512, tb * 512 + 512)
                xts = []
                for cc in range(NCC):
                    xt = xin.tile([128, 512], BF16, tag="xt", name=f"xt{tb}_{cc}")
                    nc.sync.dma_start(out=xt[0:64, :], in_=xT[0:64, cc, ts_])
                    nc.sync.dma_start(out=xt[64:128, :], in_=xT[64:128, cc, ts_])
                    xts.append(xt)
                return xts

            xts0 = load_x(0)

            for pg in range(4):
                pgs = slice(32 * pg, 32 * pg + 32)
                nc.sync.dma_start(
                    out=wk_sb[pgs].rearrange("p cc d -> p (cc d)"), in_=wkT[pgs])
                nc.sync.dma_start(
                    out=wv_sb[pgs].rearrange("p cc d -> p (cc d)"), in_=wvT[pgs])
            nc.sync.dma_start(out=mk_sb[:].rearrange("p s q -> p (s q)"), in_=maskd)

            ones_sb = consts.tile([128, NTT], F32, tag="ones")
            nc.vector.memset(ones_sb[:], 1.0)
            for h in range(HPC):
                nc.vector.tensor_copy(
                    out=v_sb[:, :, h * 65 + 64 : h * 65 + 65],
                    in_=ones_sb[:].rearrange("p (t o) -> p t o", o=1),
                )

            # ---- QKV projection for one t-block --------------------------
            # QT[d, t] = sum_c wqT[c, d] * xT[c, t]; V[t, d] = x @ Wv_sel
            def qkv(tb, xts):
                ts_ = slice(tb * 512, tb * 512 + 512)
                pq = pa.tile([128, 1024], F32, tag="pa", name="pq")
                pk = pa.tile([128, 1024], F32, tag="pa", name="pk")
                pv = pa.tile([128, 1024], F32, tag="pa", name="pv")
                for cc in range(NCC):
                    xt = xts[cc]
                    st = dict(start=(cc == 0), stop=(cc == NCC - 1))
                    nc.tensor.matmul(pq[:, 0:512], wq_sb[:, cc, 0:128], xt[:], **st)
                    nc.tensor.matmul(pq[:, 512:1024], wq_sb[:, cc, 128:256], xt[:], **st)
                    nc.tensor.matmul(pk[:, 0:512], wk_sb[:, cc, 0:128], xt[:], **st)
                    nc.tensor.matmul(pk[:, 512:1024], wk_sb[:, cc, 128:256], xt[:], **st)
                    for tt4 in range(4):
                        # two 256-col regions share a PSUM bank: only the
                        # first toucher of a bank may set start, the last stop
                        nc.tensor.matmul(
                            pv[:, tt4 * 256 : tt4 * 256 + 256],
                            xt[:, tt4 * 128 : tt4 * 128 + 128],
                            wv_sb[:, cc, :],
                            start=(cc == 0 and tt4 % 2 == 0),
                            stop=(cc == NCC - 1 and tt4 % 2 == 1),
                        )
                # PSUM -> SBUF (casts to bf16); qt on ACT so attn's first
                # S-matmul (which needs qt) unblocks while DVE does kt/v
                if qt_act:
                    nc.scalar.copy(
                        qt_sb[:, :, ts_], pq[:].rearrange("p (s t) -> p s t", s=2)
                    )
                else:
                    nc.vector.tensor_copy(
                        out=qt_sb[:, :, ts_],
                        in_=pq[:].rearrange("p (s t) -> p s t", s=2),
                    )
                nc.vector.tensor_copy(
                    out=kt_sb[:, :, ts_], in_=pk[:].rearrange("p (s t) -> p s t", s=2)
                )
                pv3 = pv[:].rearrange("p (tt d) -> p tt d", tt=4)
                for h in range(HPC):
                    nc.vector.tensor_copy(
                        out=v_sb[:, tb * 4 : tb * 4 + 4, h * 65 : h * 65 + 64],
                        in_=pv3[:, :, h * 64 : h * 64 + 64],
                    )

            # ---- attention for one (head-pair, q-block) ------------------
            psq = {}

            def attn(hp, qb):
                qs = slice(qb * 512, qb * 512 + 512)
                n_kt = 4 * (qb + 1)      # k-tiles (128) up to the diagonal
                # po lives in its own 2-bank pool: it is held across the
                # whole k-loop, so putting it in the rotating sg pool would
                # deadlock the slot rotation (S-matmul waiting on po's
                # evacuation which waits on later matmuls)
                po = pb.tile([128, 1024], F32, tag="pb", name="po")
                sgs = {}

                def emit_s(kt):
                    sg = pa.tile([128, 1024], F32, tag="pa", name=f"sg{kt%3}")
                    for s in range(2):
                        psl = slice(64 * s, 64 * s + 64)
                        nc.tensor.matmul(
                            sg[:, 512 * s : 512 * s + 512],
                            kt_sb[psl, hp, kt * 128 : kt * 128 + 128],
                            qt_sb[psl, hp, qs],
                            start=True, stop=True,
                            tile_position=(64 * s, 0),
                        )
                    sgs[kt] = sg

                emit_s(0)
                if n_kt > 1:
                    emit_s(1)
                for kt in range(n_kt):
                    if kt + 2 < n_kt:
                        emit_s(kt + 2)
                    j = kt - 4 * qb      # diag offset; >= 0 on diagonal tiles
                    roff = 128 * j if j > 0 else 0
                    sg = sgs.pop(kt)
                    pt = ptpool.tile([128, 1024], ATT_DT, tag="pt", name="pt")
                    sgv = sg[:].rearrange("p (s q) -> p s q", s=2)
                    ptv = pt[:].rearrange("p (s q) -> p s q", s=2)
                    # exp (no max subtraction; scores bounded); columns left
                    # of the diagonal block are fully masked -> skip them
                    eoff = roff if exp_trim else 0
                    nc.scalar.activation(
                        out=ptv[:, :, eoff:512], in_=sgv[:, :, eoff:512],
                        func=mybir.ActivationFunctionType.Exp,
                        scale=scale,
                    )
                    if j >= 0:
                        # triangular mask on the diagonal block, both slots
                        nc.vector.tensor_mul(
                            ptv[:, :, 128 * j : 128 * j + 128],
                            ptv[:, :, 128 * j : 128 * j + 128],
                            mk_sb[:],
                        )
                    if pv_split:
                        # P@V': K split into two row-halves so the (slot,
                        # half) matmuls pair on disjoint PE row groups
                        for s, hh in ((0, 0), (1, 1), (0, 1), (1, 0)):
                            hsl = slice(64 * hh, 64 * hh + 64)
                            head = 2 * hp + s
                            nc.tensor.matmul(
                                po[0:65, 512 * s + roff : 512 * s + 512],
                                v_sb[hsl, kt, head * 65 : head * 65 + 65],
                                pt[hsl, 512 * s + roff : 512 * s + 512],
                                start=(kt == 0 and hh == (0 if s == 0 else 1)),
                                stop=(kt == n_kt - 1 and hh == (1 if s == 0 else 0)),
                                tile_position=(64 * hh, 0),
                            )
                    else:
                        for s in range(2):
                            head = 2 * hp + s
                            nc.tensor.matmul(
                                po[0:65, 512 * s + roff : 512 * s + 512],
                                v_sb[:, kt, head * 65 : head * 65 + 65],
                                pt[:, 512 * s + roff : 512 * s + 512],
                                start=(kt == 0),
                                stop=(kt == n_kt - 1),
                            )
                # copy O^T + denominator row out of PSUM; normalization is
                # deferred (one batched reciprocal serves the whole q-block)
                for s in range(2):
                    ps_sb = norm.tile([65, 512], F32, tag="ps",
                                      name=f"ps{hp}{qb}{s}")
                    nc.vector.tensor_copy(
                        out=ps_sb[:], in_=po[0:65, s * 512 : s * 512 + 512])
                    idx = 2 * hp + s
                    # DMA may read/write any partition row (engines cannot)
                    nc.sync.dma_start(out=den[qb][idx : idx + 1, :],
                                      in_=ps_sb[64:65, :])
                    psq[(hp, qb, s)] = ps_sb

            # ---- normalization for one q-block ---------------------------
            def norm_qb(qb):
                qs = slice(qb * 512, qb * 512 + 512)
                nc.vector.reciprocal_approx_fast(rec[qb][:], den[qb][:])
                for hp2 in range(2):
                    for s in range(2):
                        idx = 2 * hp2 + s
                        rc = norm.tile([1, 512], F32, tag="rc", name="rc")
                        rb = norm.tile([64, 512], F32, tag="rb", name="rb")
                        nc.sync.dma_start(out=rc[:], in_=rec[qb][idx : idx + 1, :])
                        nc.gpsimd.partition_broadcast(rb[:], rc[:])
                        nc.vector.tensor_mul(
                            ot_sb[64 * s : 64 * s + 64, hp2, qs],
                            psq[(hp2, qb, s)][0:64, :],
                            rb[:],
                        )

            # ---- output projection for one q-block -----------------------
            def proj(qb):
                for tt in range(4 * qb, 4 * qb + 4):
                    tloc = slice(tt * 128, tt * 128 + 128)
                    pc = pa.tile([128, 1024], F32, tag="pa", name="pc")
                    for cb in range(2):
                        for hpp in range(2):
                            nc.tensor.matmul(
                                pc[:, cb * 512 : cb * 512 + 512],
                                ot_sb[:, hpp, tloc],
                                wp_sb[:, hpp, cb * 512 : cb * 512 + 512],
                                start=(hpp == 0),
                                stop=(hpp == 1),
                            )
                    ob = outst.tile([128, 1024], F16, tag="ob")
                    nc.vector.tensor_copy(out=ob[:], in_=pc[:])
                    for pg in range(2):
                        eng = nc.sync if (tt + pg) % 2 else nc.scalar
                        eng.dma_start(
                            out=out_p[tt * 128 + 64 * pg : tt * 128 + 64 * pg + 64, :],
                            in_=ob[64 * pg : 64 * pg + 64, :])

            # ---- main schedule -------------------------------------------
            qkv(0, xts0)
            for pg in range(4):
                pgs = slice(32 * pg, 32 * pg + 32)
                nc.sync.dma_start(
                    out=wp_sb[pgs].rearrange("p h c -> p (h c)"), in_=wpT[pgs])
            xts_next = load_x(1)
            for qb in range(NQB):
                attn(0, qb)
                attn(1, qb)
                if qb < NQB - 1:
                    qkv(qb + 1, xts_next)
                    if qb < NQB - 2:
                        xts_next = load_x(qb + 2)
                norm_qb(qb)
                proj(qb)

    nc.compile()
    return nc


_NC_CACHE = None


def _get_program():
    global _NC_CACHE
    if _NC_CACHE is None:
        _NC_CACHE = build_program()
    return _NC_CACHE


def make_in_maps(x, Wq, Wk, Wv, Wp):
    import ml_dtypes
    x = np.asarray(x, np.float32)
    Wq = np.asarray(Wq, np.float32)
    Wk = np.asarray(Wk, np.float32)
    Wv = np.asarray(Wv, np.float32)
    Wp = np.asarray(Wp, np.float32)
    mask1 = np.triu(np.ones((128, 128), np.float32))  # mask[k,q] = (k <= q)
    maskd = np.concatenate([mask1, mask1], axis=1).astype(ml_dtypes.bfloat16)
    in_maps = []
    for core in range(NCORES):
        b, hg = core // 4, core % 4
        sel = slice(hg * DSEL, (hg + 1) * DSEL)
        # SBUF images: [128, cc, ...] with partition index innermost in
        # the original feature dim (feature c -> (cc, p))
        xi = x[b].T.reshape(NCC, 128, T).transpose(1, 0, 2)          # [128, cc, T]
        wqi = Wq[sel, :].T.reshape(NCC, 128, DSEL).transpose(1, 0, 2).reshape(128, NCC * DSEL)
        wki = Wk[sel, :].T.reshape(NCC, 128, DSEL).transpose(1, 0, 2).reshape(128, NCC * DSEL)
        wvi = Wv[sel, :].T.reshape(NCC, 128, DSEL).transpose(1, 0, 2).reshape(128, NCC * DSEL)
        wpi = Wp[:, sel].T.reshape(2, 128, C).transpose(1, 0, 2).reshape(128, 2 * C)
        in_maps.append({
            "xT": np.ascontiguousarray(xi.astype(ml_dtypes.bfloat16)),
            "wqT": np.ascontiguousarray(wqi.astype(ml_dtypes.bfloat16)),
            "wkT": np.ascontiguousarray(wki.astype(ml_dtypes.bfloat16)),
            "wvT": np.ascontiguousarray(wvi.astype(ml_dtypes.bfloat16)),
            "wpT": np.ascontiguousarray(wpi.astype(ml_dtypes.bfloat16)),
            "maskd": maskd,
        })
    return in_maps


def combine_outputs(results, bp):
    parts = [np.asarray(results[i]["out_p"], np.float32) for i in range(NCORES)]
    out = np.stack([
        parts[0] + parts[1] + parts[2] + parts[3],
        parts[4] + parts[5] + parts[6] + parts[7],
    ])
    return (out + np.asarray(bp, np.float32)).astype(np.float32)


def kernel(x, Wq, Wk, Wv, Wp, bp):
    nc = _get_program()
    in_maps = make_in_maps(x, Wq, Wk, Wv, Wp)
    res = bass_utils.run_bass_kernel_spmd(nc, in_maps, core_ids=list(range(NCORES)))
    return combine_outputs(res.results, bp)


# revision 32
# speedup vs baseline: 1.1490x; 1.0698x over previous
# Bass/Tile Trainium2 kernel for batched multi-head causal self-attention.
#
# Problem: x[B=2,T=2048,C=1024], 16 heads (hd=64), causal softmax attention,
# output projection. Full (unsharded) inputs in, full output out.
#
# Sharding (Megatron-style): 8 cores = 2 batch groups x 4 head groups.
# Core i handles batch b = i // 4 and heads [4*(i%4) : 4*(i%4)+4).
# Each core computes Q/K/V projections for its 4 heads, causal attention,
# and a partial output projection (contribution of its heads).  The host
# sums the 4 partials per batch (the Megatron all-reduce) and adds bias.
#
# v3 structure (fully software-pipelined, QKV runs one q-block ahead):
#   qkv(0); qkv(1); for qb: attn(hp0,qb), attn(hp1,qb), qkv(qb+2),
#   norm(qb), proj(qb).  The attention exp work (ACT engine) overlaps the
#   QKV/proj matmuls of neighboring blocks, normalization is DMA-free and
#   hides under QKV, and the output DMA is spread across the whole kernel.
#
# On-device layout notes:
#   - Feature dims on partitions: xT [C, T], QT/KT [64, T] per head, with
#     head pairs packed on the 128 partitions (even head at [0:64], odd at
#     [64:128]).  S^T tiles are built with K=64 matmuls using explicit
#     tile_position row groups; the two slots run concurrently on disjoint
#     PE row halves.
#   - sg PSUM tile = [128 k, slot0 512q | slot1 512q] for ONE k-tile: one
#     [128,1024] exp per k-tile serves both heads of the pair, and the
#     pipeline (depth 2 + po accumulator) exactly fills the 8 PSUM banks.
#   - V' = [V | ones | 0pad] per head (stride 96) so the P@V matmul also
#     yields the softmax denominator (row 64), and the 96-col (32-aligned)
#     weight lets P@V split into two K=64 halves paired on disjoint PE row
#     groups (2x concurrency).  M=65 with tile_position hangs the HW.
#   - exp skips the fully-masked column range on diagonal k-tiles (strided
#     AP over both slots); a [128,2,128] strided multiply applies the
#     triangular mask to both slots at once.
#   - Normalization is DMA-free: reciprocal_approx_fast reads the
#     denominator row straight out of PSUM into one row of rcp4, a K=4
#     selector matmul broadcasts the 4 reciprocal rows onto all 128
#     partitions (s0 rows 0:64, s1 rows 64:128 = OT's layout), and one
#     [128,512] DVE multiply per head-pair writes normalized bf16 OT.
#   - Projection runs in bf16; partial outputs are cast to fp16
#     (alternating DVE/ACT) and DMA'd out incrementally; host sums the
#     4 partials per batch and adds the bias.

import numpy as np

import concourse.bass as bass
import concourse.tile as tile
from concourse import bacc, mybir
from concourse import bass_utils

F32 = mybir.dt.float32
F16 = mybir.dt.float16
BF16 = mybir.dt.bfloat16
ATT_DT = BF16

B, T, C, H = 2, 2048, 1024, 16
HD = C // H            # 64 head dim
NCORES = 8
HPC = 4                # heads per core
DSEL = HPC * HD        # 256 feature dims per core
NTT = T // 128         # 16 t-tiles of 128
NCC = C // 128         # 8 c-chunks of 128
NQB = T // 512         # 4 q-blocks of 512
VW = 96                # V' stride/head: [V(64) | ones | zeros], 32-aligned


def build_program(norm_mode='inl', pv_split=False):
    nc = bacc.Bacc("TRN2", target_bir_lowering=False, debug=False)

    xT = nc.dram_tensor("xT", [128, NCC, T], BF16, kind="ExternalInput").ap()
    wqT = nc.dram_tensor("wqT", [128, NCC * DSEL], BF16, kind="ExternalInput").ap()
    wkT = nc.dram_tensor("wkT", [128, NCC * DSEL], BF16, kind="ExternalInput").ap()
    wvT = nc.dram_tensor("wvT", [128, NCC * DSEL], BF16, kind="ExternalInput").ap()
    wpT = nc.dram_tensor("wpT", [128, 2 * C], BF16, kind="ExternalInput").ap()
    maskd = nc.dram_tensor("maskd", [128, 256], ATT_DT, kind="ExternalInput").ap()
    selc = nc.dram_tensor("selc", [97, 256], BF16, kind="ExternalInput").ap()
    out_p = nc.dram_tensor("out_p", [T, C], F16, kind="ExternalOutput").ap()

    scale = 1.0 / float(np.sqrt(HD))

    with tile.TileContext(nc) as tc:
        with (
            tc.tile_pool(name="consts", bufs=1) as consts,
            tc.tile_pool(name="persist", bufs=1) as persist,
            tc.tile_pool(name="xin0", bufs=16) as xin0,
            tc.tile_pool(name="xin", bufs=2) as xin,
            tc.tile_pool(name="pt", bufs=6) as ptpool,
            tc.tile_pool(name="norm", bufs=10) as norm,
            tc.tile_pool(name="outst", bufs=4) as outst,
            tc.tile_pool(name="pa", bufs=3, space="PSUM") as pa,
            tc.tile_pool(name="pb", bufs=1, space="PSUM") as pb,
        ):
            # ---- constants / weights -------------------------------------
            wq_sb = consts.tile([128, NCC, DSEL], BF16, tag="wq")
            wk_sb = consts.tile([128, NCC, DSEL], BF16, tag="wk")
            wv_sb = consts.tile([128, NCC, DSEL], BF16, tag="wv")
            wp_sb = consts.tile([128, 2, C], BF16, tag="wp")
            mk_sb = consts.tile([128, 2, 128], ATT_DT, tag="mk")
            sel_sb = consts.tile([97, 256], BF16, tag="sel")
            # tb0 x: fine-grained per-cc tiles so the first matmuls only
            # wait for one 128KB slice; wv/wq DMAs interleave with the
            # first x chunks so the pv-first matmuls can start early
            xts0 = []
            for cc in range(NCC):
                xt = xin0.tile([128, 512], BF16, tag="xt0", name=f"x0_{cc}")
                xts0.append(xt)
            for pg in range(4):
                pgs = slice(32 * pg, 32 * pg + 32)
                nc.sync.dma_start(
                    out=wv_sb[pgs].rearrange("p cc d -> p (cc d)"), in_=wvT[pgs])
                cc = pg
                nc.scalar.dma_start(out=xts0[cc][0:64, :], in_=xT[0:64, cc, 0:512])
                nc.scalar.dma_start(out=xts0[cc][64:128, :], in_=xT[64:128, cc, 0:512])
            for pg in range(4):
                pgs = slice(32 * pg, 32 * pg + 32)
                nc.sync.dma_start(
                    out=wq_sb[pgs].rearrange("p cc d -> p (cc d)"), in_=wqT[pgs])
                cc = 4 + pg
                nc.scalar.dma_start(out=xts0[cc][0:64, :], in_=xT[0:64, cc, 0:512])
                nc.scalar.dma_start(out=xts0[cc][64:128, :], in_=xT[64:128, cc, 0:512])

            for pg in range(4):
                pgs = slice(32 * pg, 32 * pg + 32)
                nc.sync.dma_start(
                    out=wk_sb[pgs].rearrange("p cc d -> p (cc d)"), in_=wkT[pgs])
            nc.sync.dma_start(out=mk_sb[:].rearrange("p s q -> p (s q)"), in_=maskd)
            if norm_mode == "sel":
                nc.sync.dma_start(out=sel_sb[:], in_=selc)

            def load_x(tb):
                ts_ = slice(tb * 512, tb * 512 + 512)
                xts = []
                for cc in range(NCC):
                    xt = xin0.tile([128, 512], BF16, tag="xt0", name=f"x{tb}_{cc}")
                    nc.sync.dma_start(out=xt[0:64, :], in_=xT[0:64, cc, ts_])
                    nc.scalar.dma_start(out=xt[64:128, :], in_=xT[64:128, cc, ts_])
                    xts.append(xt)
                return xts

            xts1 = load_x(1)

            # ---- persistent activations ----------------------------------
            qt_sb = persist.tile([128, 2, T], ATT_DT, tag="qt")
            kt_sb = persist.tile([128, 2, T], ATT_DT, tag="kt")
            ot_sb = persist.tile([128, 2, T], BF16, tag="ot")
            v_sb = persist.tile([128, NTT, HPC * VW], ATT_DT, tag="v")
            # per-qb reciprocal rows at 32-aligned partitions 32*idx
            # (engine APs require 32-aligned partition bases); the unused
            # rows are zeroed once so the K=97 selector matmul can read them
            if norm_mode == "sel":
                rcp4 = [persist.tile([128, 512], F32, tag=f"rcp{i}",
                                     name=f"rcp{i}") for i in range(NQB)]
                for i in range(NQB):
                    nc.vector.memset(rcp4[i][:], 0.0)
            if norm_mode == "v2":
                den = [persist.tile([4, 512], F32, tag=f"den{i}", name=f"den{i}")
                       for i in range(NQB)]
                rec = [persist.tile([4, 512], F32, tag=f"rec{i}", name=f"rec{i}")
                       for i in range(NQB)]

            ones_sb = consts.tile([128, NTT], F32, tag="ones")
            ones1_sb = consts.tile([1, 64], BF16, tag="ones1")
            nc.vector.memset(ones1_sb[:], 1.0)
            nc.vector.memset(ones_sb[:], 1.0)
            for h in range(HPC):
                nc.vector.memset(v_sb[:, :, h * VW + 65 : h * VW + VW], 0.0)
                nc.vector.tensor_copy(
                    out=v_sb[:, :, h * VW + 64 : h * VW + 65],
                    in_=ones_sb[:].rearrange("p (t o) -> p t o", o=1),
                )

            # ---- QKV projection for one t-block --------------------------
            def qkv(tb, xts):
                ts_ = slice(tb * 512, tb * 512 + 512)
                pq = pa.tile([128, 1024], F32, tag="pa", name="pq")
                pk = pa.tile([128, 1024], F32, tag="pa", name="pk")
                pv = pa.tile([128, 1024], F32, tag="pa", name="pv")
                for cc in range(NCC):
                    xt = xts[cc]
                    for tt4 in range(4):
                        # two 256-col regions share a PSUM bank: only the
                        # first toucher of a bank may set start, the last stop
                        nc.tensor.matmul(
                            pv[:, tt4 * 256 : tt4 * 256 + 256],
                            xt[:, tt4 * 128 : tt4 * 128 + 128],
                            wv_sb[:, cc, :],
                            start=(cc == 0 and tt4 % 2 == 0),
                            stop=(cc == NCC - 1 and tt4 % 2 == 1),
                        )
                for cc in range(NCC):
                    xt = xts[cc]
                    st = dict(start=(cc == 0), stop=(cc == NCC - 1))
                    nc.tensor.matmul(pq[:, 0:512], wq_sb[:, cc, 0:128], xt, **st)
                    nc.tensor.matmul(pq[:, 512:1024], wq_sb[:, cc, 128:256], xt, **st)
                for cc in range(NCC):
                    xt = xts[cc]
                    st = dict(start=(cc == 0), stop=(cc == NCC - 1))
                    nc.tensor.matmul(pk[:, 0:512], wk_sb[:, cc, 0:128], xt, **st)
                    nc.tensor.matmul(pk[:, 512:1024], wk_sb[:, cc, 128:256], xt, **st)
                # PSUM -> SBUF (casts to bf16); qt on ACT so the dependent
                # S-matmuls unblock while DVE does kt/v
                pv3 = pv[:].rearrange("p (tt d) -> p tt d", tt=4)
                for h in range(HPC):
                    nc.vector.tensor_copy(
                        out=v_sb[:, tb * 4 : tb * 4 + 4, h * VW : h * VW + 64],
                        in_=pv3[:, :, h * 64 : h * 64 + 64],
                    )
                nc.scalar.copy(
                    qt_sb[:, :, ts_], pq[:].rearrange("p (s t) -> p s t", s=2)
                )
                nc.vector.tensor_copy(
                    out=kt_sb[:, :, ts_], in_=pk[:].rearrange("p (s t) -> p s t", s=2)
                )

            # ---- attention for one (head-pair, q-block) ------------------
            psq = {}
            rcps = {}
            rbs = {}

            def attn(hp, qb):
                qs = slice(qb * 512, qb * 512 + 512)
                n_kt = 4 * (qb + 1)      # k-tiles (128) up to the diagonal
                # po lives in its own pool (held across the whole k-loop;
                # the rotating pool would deadlock); one tile per slot so
                # the next head-pair's PV only waits on one evacuation
                po = [pb.tile([128, 512], F32, tag=f"pb{s}", name=f"po{s}")
                      for s in range(2)]
                sgs = {}

                def emit_s(kt):
                    sg = pa.tile([128, 1024], F32, tag="pa", name=f"sg{kt % 3}")
                    for s in range(2):
                        psl = slice(64 * s, 64 * s + 64)
                        nc.tensor.matmul(
                            sg[:, 512 * s : 512 * s + 512],
                            kt_sb[psl, hp, kt * 128 : kt * 128 + 128],
                            qt_sb[psl, hp, qs],
                            start=True, stop=True,
                            tile_position=(64 * s, 0),
                        )
                    sgs[kt] = sg

                emit_s(0)
                emit_s(1)
                for kt in range(n_kt):
                    if kt + 2 < n_kt:
                        emit_s(kt + 2)
                    j = kt - 4 * qb      # diag offset; >= 0 on diagonal tiles
                    roff = 128 * j if j > 0 else 0
                    sg = sgs.pop(kt)
                    pt = ptpool.tile([128, 1024], ATT_DT, tag="pt", name="pt")
                    sgv = sg[:].rearrange("p (s q) -> p s q", s=2)
                    ptv = pt[:].rearrange("p (s q) -> p s q", s=2)
                    # exp (no max subtraction; scores bounded); columns left
                    # of the diagonal block are fully masked -> skip them
                    nc.scalar.activation(
                        out=ptv[:, :, roff:512], in_=sgv[:, :, roff:512],
                        func=mybir.ActivationFunctionType.Exp,
                        scale=scale,
                    )
                    if j >= 0:
                        # triangular mask on the diagonal block, both slots
                        nc.vector.tensor_mul(
                            ptv[:, :, 128 * j : 128 * j + 128],
                            ptv[:, :, 128 * j : 128 * j + 128],
                            mk_sb[:],
                        )
                    # P@V': K split into two row-halves so the (slot, half)
                    # matmuls pair on disjoint PE row groups
                    if pv_split:
                        for s, hh in ((0, 0), (1, 1), (0, 1), (1, 0)):
                            hsl = slice(64 * hh, 64 * hh + 64)
                            head = 2 * hp + s
                            nc.tensor.matmul(
                                po[0:VW, 512 * s + roff : 512 * s + 512],
                                v_sb[hsl, kt, head * VW : head * VW + VW],
                                pt[hsl, 512 * s + roff : 512 * s + 512],
                                start=(kt == 0 and hh == (0 if s == 0 else 1)),
                                stop=(kt == n_kt - 1 and hh == (1 if s == 0 else 0)),
                                tile_position=(64 * hh, 0),
                            )
                    else:
                        for s in range(2):
                            head = 2 * hp + s
                            nc.tensor.matmul(
                                po[s][0:65, roff:512],
                                v_sb[:, kt, head * VW : head * VW + 65],
                                pt[:, 512 * s + roff : 512 * s + 512],
                                start=(kt == 0),
                                stop=(kt == n_kt - 1),
                            )
                # O^T + denominator row out of PSUM, then normalize this
                # head-pair right away: reciprocal from the SBUF den row,
                # GpSimd partition-broadcast, DVE multiply into bf16 OT.
                # The chain is DMA-free so the last q-block's norm is short.
                tailhp = norm_mode == "inl" and hp == 1 and qb == NQB - 1
                for s in range(2):
                    ps_sb = norm.tile([65, 512], F32, tag="ps",
                                      name=f"ps{hp}{qb}{s}")
                    # in the tail, evacs run on both engines in parallel
                    if tailhp and s == 0:
                        nc.scalar.copy(ps_sb[:], po[s][0:65, :])
                    else:
                        nc.vector.tensor_copy(out=ps_sb[:], in_=po[s][0:65, :])
                    psq[(hp, qb, s)] = ps_sb
                if norm_mode == "inl":
                    for s in range(2):
                        dsb = norm.tile([1, 512], F32, tag="dsb", name="dsb")
                        rcp = norm.tile([1, 512], F32, tag="rcp", name="rcp")
                        # den row straight out of PSUM (plain copy; only the
                        # custom DVE op mis-reads partition-base-64 APs)
                        nc.vector.tensor_copy(out=dsb[:], in_=po[s][64:65, :])
                        nc.vector.reciprocal_approx_fast(rcp[:], dsb[:])
                        if hp == 1 and qb == NQB - 1:
                            # tail: broadcast via K=1 PE matmul + chunked
                            # muls interleaved with proj; bf16 so the
                            # matmul's moving operand runs at full rate
                            rcpb = norm.tile([1, 512], BF16, tag="rcpb",
                                             name="rcpb")
                            nc.vector.tensor_copy(out=rcpb[:], in_=rcp[:])
                            rcps[s] = rcpb
                            continue
                        rb = norm.tile([64, 512], F32, tag="rb", name="rb")
                        nc.gpsimd.partition_broadcast(rb[:], rcp[:])
                        nc.vector.tensor_mul(
                            ot_sb[64 * s : 64 * s + 64, hp, qs],
                            psq[(hp, qb, s)][0:64, :],
                            rb[:],
                        )
                        rbs[(hp, s)] = rb
                else:
                    for s in range(2):
                        idx = 2 * hp + s
                        nc.sync.dma_start(out=den[qb][idx : idx + 1, :],
                                          in_=psq[(hp, qb, s)][64:65, :])

            # ---- normalization for one q-block (DMA-free) ----------------
            def norm_qb(qb):
                if norm_mode == "inl":
                    return
                qs = slice(qb * 512, qb * 512 + 512)
                if norm_mode == "v2":
                    nc.vector.reciprocal_approx_fast(rec[qb][:], den[qb][:])
                    for hp2 in range(2):
                        for s in range(2):
                            idx = 2 * hp2 + s
                            rc = norm.tile([1, 512], F32, tag="rc", name="rc")
                            rb = norm.tile([64, 512], F32, tag="rb", name="rb")
                            nc.sync.dma_start(out=rc[:],
                                              in_=rec[qb][idx : idx + 1, :])
                            nc.gpsimd.partition_broadcast(rb[:], rc[:])
                            nc.vector.tensor_mul(
                                ot_sb[64 * s : 64 * s + 64, hp2, qs],
                                psq[(hp2, qb, s)][0:64, :],
                                rb[:],
                            )
                    return
                if norm_mode == "sel":
                    rcb = norm.tile([128, 512], BF16, tag="rcb", name="rcb")
                    nc.vector.tensor_copy(out=rcb[:], in_=rcp4[qb][:])
                    rb2 = pa.tile([128, 1024], F32, tag="pa", name="rb2")
                    for hp2 in range(2):
                        # rb2[m, q] = rcb[32*(2*hp2 + (m>=64)), q]: recip
                        # rows broadcast to OT's layout via K=97 matmul
                        nc.tensor.matmul(
                            rb2[:, hp2 * 512 : hp2 * 512 + 512],
                            sel_sb[:, hp2 * 128 : hp2 * 128 + 128],
                            rcb[0:97, :],
                            start=True, stop=True,
                        )
                    for hp2 in range(2):
                        nc.vector.tensor_mul(
                            ot_sb[:, hp2, qs],
                            psq[(hp2, qb)][:],
                            rb2[:, hp2 * 512 : hp2 * 512 + 512],
                        )
                else:  # norm_mode == "gps"
                    for hp2 in range(2):
                        for s in range(2):
                            idx = 2 * hp2 + s
                            rb = norm.tile([64, 512], F32, tag="rb", name="rb")
                            nc.gpsimd.partition_broadcast(
                                rb[:], rcp4[qb][32 * idx : 32 * idx + 1, :])
                            nc.vector.tensor_mul(
                                ot_sb[64 * s : 64 * s + 64, hp2, qs],
                                psq[(hp2, qb)][64 * s : 64 * s + 64, :],
                                rb[:],
                            )

            # ---- output projection for one q-block -----------------------
            def proj_tt(tt):
                    qb = tt // 4
                    tloc = slice(tt * 128, tt * 128 + 128)
                    pc = pa.tile([128, 1024], F32, tag="pa", name="pc")
                    # bank-alternating order (A,B,A,B): back-to-back
                    # accumulating matmuls to the same bank serialize on
                    # the drain, alternating hides it
                    for hpp in range(2):
                        for cb in range(2):
                            nc.tensor.matmul(
                                pc[:, cb * 512 : cb * 512 + 512],
                                ot_sb[:, hpp, tloc],
                                wp_sb[:, hpp, cb * 512 : cb * 512 + 512],
                                start=(hpp == 0),
                                stop=(hpp == 1),
                            )
                    ob = outst.tile([128, 1024], F16, tag="ob")
                    if qb == NQB - 1:
                        nc.scalar.copy(ob[:], pc[:])  # ACT is idle in the tail
                    else:
                        nc.vector.tensor_copy(out=ob[:], in_=pc[:])
                    for pg in range(4):
                        eng = nc.sync if (tt + pg) % 2 else nc.scalar
                        eng.dma_start(
                            out=out_p[tt * 128 + 32 * pg : tt * 128 + 32 * pg + 32, :],
                            in_=ob[32 * pg : 32 * pg + 32, :])

            def proj(qb):
                for tt in range(4 * qb, 4 * qb + 4):
                    proj_tt(tt)

            # ---- main schedule (QKV runs one q-block ahead) --------------
            qkv(0, xts0)
            for pg in range(4):
                pgs = slice(32 * pg, 32 * pg + 32)
                nc.sync.dma_start(
                    out=wp_sb[pgs].rearrange("p h c -> p (h c)"), in_=wpT[pgs])
            qkv(1, xts1)
            xts2 = load_x(2)
            attn(0, 0)
            attn(1, 0)
            qkv(2, xts2)
            xts3 = load_x(3)
            norm_qb(0)
            proj(0)
            attn(0, 1)
            attn(1, 1)
            qkv(3, xts3)
            norm_qb(1)
            proj(1)
            attn(0, 2)
            attn(1, 2)
            norm_qb(2)
            attn(0, 3)      # big block pulled ahead so proj(2) and the
            proj(2)         # qb2 norm chain hide under its matmuls
            attn(1, 3)
            if norm_mode != "inl":
                norm_qb(3)
                proj(3)
            else:
                # tail: hp1 reciprocals broadcast on the (idle) PE via K=1
                # ones-matmul, then per-t-tile chunked muls feed each
                # projection tile as soon as its OT slice is normalized
                qb = NQB - 1
                qs0 = qb * 512
                rbp = pa.tile([64, 1024], F32, tag="pa", name="rbp")
                for s in range(2):
                    nc.tensor.matmul(
                        rbp[:, s * 512 : s * 512 + 512],
                        ones1_sb[:], rcps[s][:],
                        start=True, stop=True,
                    )
                for tt in range(4 * qb, 4 * qb + 4):
                    ck = slice((tt - 4 * qb) * 128, (tt - 4 * qb) * 128 + 128)
                    for hp2 in range(2):
                        for s in range(2):
                            rbin = (rbs[(0, s)][0:64, ck] if hp2 == 0
                                    else rbp[:, s * 512 : s * 512 + 512][:, ck])
                            nc.vector.tensor_mul(
                                ot_sb[64 * s : 64 * s + 64, hp2,
                                      qs0 + ck.start : qs0 + ck.stop],
                                psq[(hp2, qb, s)][0:64, ck],
                                rbin,
                            )
                    proj_tt(tt)

    nc.compile()
    return nc


_NC_CACHE = None


def _get_program():
    global _NC_CACHE
    if _NC_CACHE is None:
        _NC_CACHE = build_program()
    return _NC_CACHE


def make_in_maps(x, Wq, Wk, Wv, Wp):
    import ml_dtypes
    x = np.asarray(x, np.float32)
    Wq = np.asarray(Wq, np.float32)
    Wk = np.asarray(Wk, np.float32)
    Wv = np.asarray(Wv, np.float32)
    Wp = np.asarray(Wp, np.float32)
    mask1 = np.triu(np.ones((128, 128), np.float32))  # mask[k,q] = (k <= q)
    maskd = np.concatenate([mask1, mask1], axis=1).astype(ml_dtypes.bfloat16)
    selc = np.zeros((97, 256), np.float32)
    for hp2 in range(2):
        for m in range(128):
            selc[32 * (2 * hp2 + (m >= 64)), hp2 * 128 + m] = 1.0
    selc = selc.astype(ml_dtypes.bfloat16)
    in_maps = []
    for core in range(NCORES):
        b, hg = core // 4, core % 4
        sel = slice(hg * DSEL, (hg + 1) * DSEL)
        # SBUF images: [128, cc, ...] with partition index innermost in
        # the original feature dim (feature c -> (cc, p))
        xi = x[b].T.reshape(NCC, 128, T).transpose(1, 0, 2)          # [128, cc, T]
        wqi = Wq[sel, :].T.reshape(NCC, 128, DSEL).transpose(1, 0, 2).reshape(128, NCC * DSEL)
        wki = Wk[sel, :].T.reshape(NCC, 128, DSEL).transpose(1, 0, 2).reshape(128, NCC * DSEL)
        wvi = Wv[sel, :].T.reshape(NCC, 128, DSEL).transpose(1, 0, 2).reshape(128, NCC * DSEL)
        wpi = Wp[:, sel].T.reshape(2, 128, C).transpose(1, 0, 2).reshape(128, 2 * C)
        in_maps.append({
            "xT": np.ascontiguousarray(xi.astype(ml_dtypes.bfloat16)),
            "wqT": np.ascontiguousarray(wqi.astype(ml_dtypes.bfloat16)),
            "wkT": np.ascontiguousarray(wki.astype(ml_dtypes.bfloat16)),
            "wvT": np.ascontiguousarray(wvi.astype(ml_dtypes.bfloat16)),
            "wpT": np.ascontiguousarray(wpi.astype(ml_dtypes.bfloat16)),
            "maskd": maskd,
            "selc": selc,
        })
    return in_maps


def combine_outputs(results, bp):
    parts = [np.asarray(results[i]["out_p"], np.float32) for i in range(NCORES)]
    out = np.stack([
        parts[0] + parts[1] + parts[2] + parts[3],
        parts[4] + parts[5] + parts[6] + parts[7],
    ])
    return (out + np.asarray(bp, np.float32)).astype(np.float32)


def kernel(x, Wq, Wk, Wv, Wp, bp):
    nc = _get_program()
    in_maps = make_in_maps(x, Wq, Wk, Wv, Wp)
    res = bass_utils.run_bass_kernel_spmd(nc, in_maps, core_ids=list(range(NCORES)))
    return combine_outputs(res.results, bp)
